# revision 12
# baseline (speedup 1.0000x reference)
"""ChebGraphConv (K=3) on 8 TRN2 NeuronCores.

out = x @ W0 + (Lx) @ W1 + (2L(Lx) - x) @ W2 + bias
    = x @ (W0 - W2) + T1 @ W1 + spmm(U) + bias
where T1 = spmm(x), U = T1 @ (2 W2)   (spmm commutes with right-matmul).

Sharding: destination nodes split 8 ways (N padded 50000 -> 50176 = 8*49*128).
Each core runs two SpMM passes over its ~200k edges.

Per 128-dest tile: gather source rows with dma_gather (int16 idx, bf16 512B
rows), build selector S[e,d] = w_e * (dest_e == d) with one tensor_scalar per
128-edge chunk, PSUM-accumulate matmuls. Phase 1 computes T1 transposed
directly (lhsT=G feature halves, rhs=S) so no PE transposes are needed.
Between passes, U (bf16) is AllGather'd (two collectives, first pipelined
behind pass 1).

dma_gather descriptor generation runs on ONE Q7 core-pair selected by
queue_num (4 SWDGE queues -> 4-way desc-gen parallelism, ~2ns/idx aggregate
vs ~8ns/idx on one queue). To keep all 4 queues busy the gather stream is
deep-pipelined: section-granular gathers into a deep buffer pool, ragged
per-tile static index counts (sections padded with index 0 / weight 0 only
up to their own 128-multiple, not a global max), strict round-robin queues.
meta/idx DRAM layouts stay row-blocked (contiguous per tile) so their loads
coalesce into spray descriptors.
"""

import numpy as np
import ml_dtypes

import concourse.bass as bass
import concourse.mybir as mybir
import concourse.tile as tile
from concourse import bacc
from concourse.bass_utils import run_bass_kernel_spmd

N = 50000
NP = 50176          # padded: 8 * 49 * 128
F = 256
P = 128
NCORE = 8
SH = NP // NCORE    # 6272 rows per core
T = SH // P         # 49 dest tiles per core
TA = 24             # tiles whose U rows go to the first AllGather
TB = T - TA         # 25
SHA = TA * P        # 3072
SHB = TB * P        # 3200
XSPLIT = NP // 2    # 25088: phase-1 gather table split (int16 range)

F32 = mybir.dt.float32
BF16 = mybir.dt.bfloat16
I16 = mybir.dt.int16
NPBF16 = ml_dtypes.bfloat16


# ---------------------------------------------------------------- host prep

def _wrap_idx(arr):
    """[n] int16 -> [128, n//16]: 16-partition wrap, replicated for 8 Q7 cores."""
    n = len(arr)
    w16 = arr.reshape(n // 16, 16).T
    return np.tile(w16, (8, 1)).copy()


def _pad_to(arr, n, fill):
    out = np.full(n, fill, arr.dtype)
    out[: len(arr)] = arr
    return out


def prepare(x, edge_row, edge_col, edge_w, weight, bias, mode="full", repeat=1,
            pool_every=1000000, nq=4, gbufs=8):
    x = np.asarray(x, np.float32)
    edge_row = np.asarray(edge_row, np.int32)
    edge_col = np.asarray(edge_col, np.int32)
    edge_w = np.asarray(edge_w, np.float32)
    weight = np.asarray(weight, np.float32)
    bias = np.asarray(bias, np.float32)

    order = np.argsort(edge_row, kind="stable")
    edge_row = edge_row[order]
    edge_col = edge_col[order]
    edge_w = edge_w[order]

    x_pad = np.zeros((NP, F), np.float32)
    x_pad[:N] = x

    bounds = np.searchsorted(edge_row, np.arange(0, NP + 1, P))

    owner = edge_col // SH
    rloc = edge_col % SH
    m1 = edge_col < XSPLIT
    i1lo = edge_col
    i1hi = edge_col - XSPLIT
    m2 = rloc < SHA
    i2lo = owner * SHA + rloc
    i2hi = owner * SHB + (rloc - SHA)

    dloc = (edge_row % P).astype(np.float32)

    def cnt_of(n):
        return max(1, -(-int(n) // P))

    # per-tile static section sizes = max over cores (one shared program)
    geom = np.zeros((NCORE, T, 4), np.int32)
    for c in range(NCORE):
        for t in range(T):
            g = c * T + t
            lo, hi = bounds[g], bounds[g + 1]
            msk1 = m1[lo:hi]
            msk2 = m2[lo:hi]
            geom[c, t] = (cnt_of(msk1.sum()), cnt_of((~msk1).sum()),
                          cnt_of(msk2.sum()), cnt_of((~msk2).sum()))
    geom_max = [tuple(int(v) for v in geom[:, t].max(axis=0))
                for t in range(T)]
    CS = max(max(gm) for gm in geom_max)
    C1W = max(gm[0] + gm[1] for gm in geom_max)   # phase-1 tile width
    C2AW = max(gm[2] for gm in geom_max)
    C2BW = max(gm[3] for gm in geom_max)

    def build_sec(vals_i, d_all, w_all, cnt):
        """One section padded to cnt chunks: meta [128, 2*cnt], idx [128, cnt*8]."""
        idx = _pad_to(vals_i.astype(np.int16), cnt * P, 0)
        d_sec = _pad_to(d_all, cnt * P, 0.0)
        w_sec = _pad_to(w_all, cnt * P, 0.0)
        meta = np.concatenate(
            [d_sec.reshape(cnt, P).T, w_sec.reshape(cnt, P).T], axis=1
        ).astype(NPBF16)
        return meta, _wrap_idx(idx)

    # fixed-width row-blocked layouts (contiguous per tile -> spray DMA);
    # sections packed at the front of each row-block, ragged sizes used
    # only for the gather instruction shapes
    meta1 = np.zeros((NCORE, T, P, 2 * C1W), NPBF16)
    idx1 = np.zeros((NCORE, T, P, C1W * 8), np.int16)
    meta2a = np.zeros((NCORE, T, P, 2 * C2AW), NPBF16)
    idx2a = np.zeros((NCORE, T, P, C2AW * 8), np.int16)
    meta2b = np.zeros((NCORE, T, P, 2 * C2BW), NPBF16)
    idx2b = np.zeros((NCORE, T, P, C2BW * 8), np.int16)

    for c in range(NCORE):
        for t in range(T):
            g = c * T + t
            lo, hi = bounds[g], bounds[g + 1]
            sel = slice(lo, hi)
            d_all = dloc[sel]
            w_all = edge_w[sel]
            msk1 = m1[sel]
            msk2 = m2[sel]
            cl, ch, ca, cb = geom_max[t]
            C = cl + ch
            mlo, xlo = build_sec(i1lo[sel][msk1], d_all[msk1],
                                 w_all[msk1], cl)
            mhi, xhi = build_sec(i1hi[sel][~msk1], d_all[~msk1],
                                 w_all[~msk1], ch)
            # combined layout: [d_lo | d_hi | w_lo | w_hi], [idx_lo | idx_hi]
            meta1[c, t, :, 0:C] = np.concatenate(
                [mlo[:, :cl], mhi[:, :ch]], axis=1)
            meta1[c, t, :, C:2 * C] = np.concatenate(
                [mlo[:, cl:], mhi[:, ch:]], axis=1)
            idx1[c, t, :, 0:C * 8] = np.concatenate([xlo, xhi], axis=1)

            ma, xa = build_sec(i2lo[sel][msk2], d_all[msk2], w_all[msk2], ca)
            meta2a[c, t, :, 0:2 * ca] = ma
            idx2a[c, t, :, 0:ca * 8] = xa
            mb, xb = build_sec(i2hi[sel][~msk2], d_all[~msk2],
                               w_all[~msk2], cb)
            meta2b[c, t, :, 0:2 * cb] = mb
            idx2b[c, t, :, 0:cb * 8] = xb

    # xT tiles: [t*128, 256]; cols k*128+j hold x_pad[tile row j, feat k*128+p]
    xl = x_pad.reshape(NCORE, T, P, F)
    xtt = np.ascontiguousarray(np.transpose(xl, (0, 1, 3, 2))).reshape(
        NCORE, T, 2, P, P)
    xtt = np.ascontiguousarray(np.transpose(xtt, (0, 1, 3, 2, 4))).reshape(
        NCORE, T, P, F)

    # weights: m0 = W0 - W2, m1 = W1, m2 = 2*W2; wpack[p, (k*3+m)*256 + j]
    wm = np.stack([weight[0] - weight[2], weight[1], 2.0 * weight[2]])
    wpack = np.empty((P, 2 * 3 * F), np.float32)
    for k in range(2):
        for m in range(3):
            wpack[:, (k * 3 + m) * F:(k * 3 + m + 1) * F] = \
                wm[m, k * P:(k + 1) * P, :]

    biasrow = bias.reshape(1, F)
    iota = np.broadcast_to(np.arange(P, dtype=np.float32), (P, P))
    x_lo = x_pad[:XSPLIT]
    x_hi = x_pad[XSPLIT:]

    nc = build_program(geom_max, CS, C1W, C2AW, C2BW, mode=mode,
                       repeat=repeat, nq=nq, gbufs=gbufs)

    in_maps = []
    for c in range(NCORE):
        in_maps.append({
            "x_lo": x_lo.astype(NPBF16),
            "x_hi": x_hi.astype(NPBF16),
            "meta1": np.ascontiguousarray(meta1[c].reshape(T * P, 2 * C1W)),
            "idx1": np.ascontiguousarray(idx1[c].reshape(T * P, C1W * 8)),
            "meta2a": np.ascontiguousarray(meta2a[c].reshape(T * P, 2 * C2AW)),
            "idx2a": np.ascontiguousarray(idx2a[c].reshape(T * P, C2AW * 8)),
            "meta2b": np.ascontiguousarray(meta2b[c].reshape(T * P, 2 * C2BW)),
            "idx2b": np.ascontiguousarray(idx2b[c].reshape(T * P, C2BW * 8)),
            "xtt": np.ascontiguousarray(xtt[c].reshape(T * P, F)).astype(NPBF16),
            "wpack": wpack.astype(NPBF16),
            "biasrow": biasrow.astype(NPBF16),
            "iota": iota.astype(NPBF16),
        })
    return nc, in_maps


def assemble(results):
    out = np.concatenate([results[c]["out"] for c in range(NCORE)], axis=0)
    return np.ascontiguousarray(out[:N])


# ---------------------------------------------------------------- device

def build_program(geom, CS, C1W, C2AW, C2BW, mode="full", repeat=1, nq=4,
                  gbufs=8):
    # geom: per-tile (c1l, c1h, c2a, c2b) static section sizes
    # mode: "full" | "p1" (phase 1 only) | "p1cc" (phase 1 + collectives)
    nc = bacc.Bacc("TRN2", target_bir_lowering=False, debug=False,
                   num_devices=NCORE, num_swdge_queues=nq)

    x_lo_d = nc.dram_tensor("x_lo", [XSPLIT, F], BF16, kind="ExternalInput")
    x_hi_d = nc.dram_tensor("x_hi", [NP - XSPLIT, F], BF16,
                            kind="ExternalInput")
    meta1_d = nc.dram_tensor("meta1", [T * P, 2 * C1W], BF16,
                             kind="ExternalInput")
    idx1_d = nc.dram_tensor("idx1", [T * P, C1W * 8], I16,
                            kind="ExternalInput")
    meta2a_d = nc.dram_tensor("meta2a", [T * P, 2 * C2AW], BF16,
                              kind="ExternalInput")
    idx2a_d = nc.dram_tensor("idx2a", [T * P, C2AW * 8], I16,
                             kind="ExternalInput")
    meta2b_d = nc.dram_tensor("meta2b", [T * P, 2 * C2BW], BF16,
                              kind="ExternalInput")
    idx2b_d = nc.dram_tensor("idx2b", [T * P, C2BW * 8], I16,
                             kind="ExternalInput")
    xtt_d = nc.dram_tensor("xtt", [T * P, F], BF16, kind="ExternalInput")
    wpack_d = nc.dram_tensor("wpack", [P, 6 * F], BF16, kind="ExternalInput")
    bias_d = nc.dram_tensor("biasrow", [1, F], BF16, kind="ExternalInput")
    iota_d = nc.dram_tensor("iota", [P, P], BF16, kind="ExternalInput")
    out_d = nc.dram_tensor("out", [SH, F], F32, kind="ExternalOutput")

    gq_counter = [0]

    with tile.TileContext(nc) as tc:
        with tc.tile_pool(name="const", bufs=1) as cp, \
             tc.tile_pool(name="mp", bufs=8) as mp, \
             tc.tile_pool(name="gp", bufs=gbufs) as gp, \
             tc.tile_pool(name="sb", bufs=3) as sb, \
             tc.tile_pool(name="sel", bufs=3) as selp, \
             tc.tile_pool(name="stash", bufs=T) as stash, \
             tc.tile_pool(name="ps", bufs=2, space="PSUM") as ps, \
             tc.tile_pool(name="dram", bufs=1, space="DRAM") as dp:

            iota_t = cp.tile([P, P], BF16, tag="iota")
            nc.sync.dma_start(out=iota_t[:], in_=iota_d[:])
            wpk = cp.tile([P, 6 * F], BF16, tag="wpk")
            nc.sync.dma_start(out=wpk[:], in_=wpack_d[:])
            bias_t = cp.tile([1, F], BF16, tag="bias")
            nc.sync.dma_start(out=bias_t[:], in_=bias_d[:])
            ones_t = cp.tile([1, P], BF16, tag="ones")
            nc.vector.memset(ones_t[:], 1.0)

            def next_q():
                q = gq_counter[0] % nq
                gq_counter[0] += 1
                return q

            def w_ap(m, k):
                return wpk[:, (k * 3 + m) * F:(k * 3 + m + 1) * F]

            CSW = max(geom_t[0] + geom_t[1] for geom_t in geom)

            def build_sel_wide(meta_t, C):
                """S[e, c, d] = (iota[d] == dest[e,c]) * w[e,c] — two wide
                tensor_tensor ops; tensor_tensor runs 1-port and never locks
                GpSimd out of SBUF (tensor_scalar's 2-port mode does, which
                starves SWDGE descriptor generation)."""
                s_w = selp.tile([P, CSW, P], BF16, tag="s")
                in0 = iota_t[:].unsqueeze(1).broadcast_to([P, C, P])
                d_bc = meta_t[:, 0:C].unsqueeze(2).broadcast_to([P, C, P])
                w_bc = meta_t[:, C:2 * C].unsqueeze(2).broadcast_to([P, C, P])
                nc.vector.tensor_tensor(out=s_w[:, 0:C, :], in0=in0, in1=d_bc,
                                        op=mybir.AluOpType.is_equal)
                nc.vector.tensor_tensor(out=s_w[:, 0:C, :], in0=s_w[:, 0:C, :],
                                        in1=w_bc, op=mybir.AluOpType.mult)
                return s_w

            def load_meta(t, meta_d, mw, idx_d, xw):
                meta_t = mp.tile([P, 2 * mw], BF16, tag="meta")
                idx_t = mp.tile([P, xw * 8], I16, tag="idx")
                nc.sync.dma_start(out=meta_t[:],
                                  in_=meta_d[t * P:(t + 1) * P, :])
                nc.sync.dma_start(out=idx_t[:],
                                  in_=idx_d[t * P:(t + 1) * P, :])
                return meta_t, idx_t

            def gather(idx_t, ioff, cnt, tab):
                g = gp.tile([P, CS, F], BF16, tag="g")
                nc.gpsimd.dma_gather(
                    out_ap=g[:, 0:cnt, :], in_ap=tab[:, :],
                    idxs_ap=idx_t[:, ioff:ioff + cnt * 8],
                    num_idxs=cnt * P, num_idxs_reg=cnt * P, elem_size=F,
                    single_packet=False, queue_num=next_q())
                return g

            for _rep in range(repeat):
                u_a = dp.tile([SHA, F], BF16, tag=f"ua{_rep}")
                u_b = dp.tile([SHB, F], BF16, tag=f"ub{_rep}")
                u_g1 = dp.tile([NCORE * SHA, F], BF16, tag=f"ug1{_rep}",
                               addr_space="Shared")
                u_g2 = dp.tile([NCORE * SHB, F], BF16, tag=f"ug2{_rep}",
                               addr_space="Shared")
                o1_tiles = []
                if mode == "p2x":
                    for t in range(T):
                        o1 = stash.tile([P, F], F32, tag="o1")
                        nc.scalar.memzero(o1[:])
                        o1_tiles.append(o1)
                # ---------------- phase 1 ----------------
                # mode g1: gathers only (Pool/DMA throughput in situ)
                # mode s1: selectors+matmuls only (compute chain in situ)
                for t in ([] if mode == "p2x" else range(T)):
                    cl, ch = geom[t][0], geom[t][1]
                    C = cl + ch
                    meta_t, idx_t = load_meta(t, meta1_d, C1W, idx1_d, C1W)
                    if mode == "g1":
                        gather(idx_t, 0, cl, x_lo_d)
                        gather(idx_t, cl * 8, ch, x_hi_d)
                        continue
                    if mode == "gs1":
                        g_lo = gather(idx_t, 0, cl, x_lo_d)
                        g_hi = gather(idx_t, cl * 8, ch, x_hi_d)
                        build_sel_wide(meta_t, C)
                        continue
                    if mode == "s1":
                        g_lo = gp.tile([P, CS, F], BF16, tag="g")
                        g_hi = gp.tile([P, CS, F], BF16, tag="g")
                        nc.vector.memset(g_lo[:, 0, 0:2], 0.0)
                        nc.vector.memset(g_hi[:, 0, 0:2], 0.0)
                    else:
                        g_lo = gather(idx_t, 0, cl, x_lo_d)
                        g_hi = gather(idx_t, cl * 8, ch, x_hi_d)
                    # T1.T accumulated directly: two [128f, 128d] PSUM tiles
                    # (separate banks — interleaved accumulation groups
                    # sharing one bank corrupt results)
                    t1T = sb.tile([P, F], BF16, tag="t1T")
                    t1a_ps = ps.tile([P, P], F32, tag="t1a")
                    t1b_ps = ps.tile([P, P], F32, tag="t1b")
                    halves = [(t1a_ps[:], slice(0, P)),
                              (t1b_ps[:], slice(P, F))]
                    s_w = build_sel_wide(meta_t, C)
                    for c in range(C):
                        g, ci = (g_lo, c) if c < cl else (g_hi, c - cl)
                        for acc_ap, fsl in halves:
                            nc.tensor.matmul(acc_ap, lhsT=g[:, ci, fsl],
                                             rhs=s_w[:, c, :],
                                             start=(c == 0), stop=(c == C - 1))
                    for acc_ap, fsl in halves:
                        nc.scalar.copy(t1T[:, fsl], acc_ap)

                    u_ps = ps.tile([P, F], F32, tag="uo2")
                    nc.tensor.matmul(u_ps[:], lhsT=t1T[:, 0:P], rhs=w_ap(2, 0),
                                     start=True, stop=False)
                    nc.tensor.matmul(u_ps[:], lhsT=t1T[:, P:F], rhs=w_ap(2, 1),
                                     start=False, stop=True)
                    u_sb = sb.tile([P, F], BF16, tag="usb")
                    nc.scalar.copy(u_sb[:], u_ps[:])
                    if t < TA:
                        nc.scalar.dma_start(out=u_a[t * P:(t + 1) * P, :],
                                            in_=u_sb[:])
                    else:
                        nc.scalar.dma_start(
                            out=u_b[(t - TA) * P:(t - TA + 1) * P, :],
                            in_=u_sb[:])

                    xt_t = sb.tile([P, F], BF16, tag="xt")
                    nc.sync.dma_start(out=xt_t[:],
                                      in_=xtt_d[t * P:(t + 1) * P, :])
                    o_ps = ps.tile([P, F], F32, tag="o")
                    nc.tensor.matmul(o_ps[:], lhsT=xt_t[:, 0:P], rhs=w_ap(0, 0),
                                     start=True, stop=False)
                    nc.tensor.matmul(o_ps[:], lhsT=xt_t[:, P:F], rhs=w_ap(0, 1),
                                     start=False, stop=False)
                    nc.tensor.matmul(o_ps[:], lhsT=t1T[:, 0:P], rhs=w_ap(1, 0),
                                     start=False, stop=False)
                    nc.tensor.matmul(o_ps[:], lhsT=t1T[:, P:F], rhs=w_ap(1, 1),
                                     start=False, stop=False)
                    nc.tensor.matmul(o_ps[:], lhsT=ones_t[:], rhs=bias_t[:],
                                     start=False, stop=True)
                    o1 = stash.tile([P, F], F32, tag="o1")
                    nc.scalar.copy(o1[:], o_ps[:])
                    o1_tiles.append(o1)

                    if t == TA - 1 and mode not in ("p1",):
                        nc.gpsimd.collective_compute(
                            "AllGather", mybir.AluOpType.bypass,
                            replica_groups=[list(range(NCORE))],
                            ins=[u_a[:].opt()], outs=[u_g1[:].opt()])
                if mode not in ("p1",):
                    nc.gpsimd.collective_compute(
                        "AllGather", mybir.AluOpType.bypass,
                        replica_groups=[list(range(NCORE))],
                        ins=[u_b[:].opt()], outs=[u_g2[:].opt()])

                # ---------------- phase 2a: a sections (need only AG1) ----
                if mode in ("full", "p2x"):
                    for t in range(T):
                        c2 = geom[t][2]
                        meta_t, idx_t = load_meta(t, meta2a_d, C2AW,
                                                  idx2a_d, C2AW)
                        g = gather(idx_t, 0, c2, u_g1)
                        o2_ps = ps.tile([P, F], F32, tag="uo2")
                        s_w = build_sel_wide(meta_t, c2)
                        for c in range(c2):
                            nc.tensor.matmul(o2_ps[:], lhsT=s_w[:, c, :],
                                             rhs=g[:, c, :],
                                             start=(c == 0),
                                             stop=(c == c2 - 1))
                        nc.vector.tensor_add(o1_tiles[t][:], o1_tiles[t][:],
                                             o2_ps[:])
                    # ------------ phase 2b: b sections (need AG2) ----------
                    for t in range(T):
                        c2 = geom[t][3]
                        meta_t, idx_t = load_meta(t, meta2b_d, C2BW,
                                                  idx2b_d, C2BW)
                        g = gather(idx_t, 0, c2, u_g2)
                        o2_ps = ps.tile([P, F], F32, tag="uo2")
                        s_w = build_sel_wide(meta_t, c2)
                        for c in range(c2):
                            nc.tensor.matmul(o2_ps[:], lhsT=s_w[:, c, :],
                                             rhs=g[:, c, :],
                                             start=(c == 0),
                                             stop=(c == c2 - 1))
                        fin = sb.tile([P, F], F32, tag="fin")
                        nc.vector.tensor_add(fin[:], o1_tiles[t][:], o2_ps[:])
                        nc.scalar.dma_start(out=out_d[t * P:(t + 1) * P, :],
                                            in_=fin[:])
                else:
                    for t in range(T):
                        fin = sb.tile([P, F], F32, tag="fin")
                        if o1_tiles:
                            nc.scalar.copy(fin[:], o1_tiles[t][:])
                        else:
                            nc.scalar.memzero(fin[:])
                        nc.scalar.dma_start(out=out_d[t * P:(t + 1) * P, :],
                                            in_=fin[:])

    nc.compile()
    return nc


# ---------------------------------------------------------------- entry

def kernel(x, edge_row, edge_col, edge_w, weight, bias):
    nc, in_maps = prepare(x, edge_row, edge_col, edge_w, weight, bias)
    res = run_bass_kernel_spmd(nc, in_maps, core_ids=list(range(NCORE)))
    return assemble(res.results)


# revision 14
# speedup vs baseline: 1.1757x; 1.1757x over previous
"""ChebGraphConv (K=3) on 8 TRN2 NeuronCores.

out = x @ W0 + (Lx) @ W1 + (2L(Lx) - x) @ W2 + bias
    = x @ (W0 - W2) + T1 @ W1 + spmm(U) + bias
where T1 = spmm(x), U = T1 @ (2 W2)   (spmm commutes with right-matmul).

Sharding: destination nodes split 8 ways (N padded 50000 -> 50176 = 8*49*128).
Each core runs two SpMM passes over its ~200k edges.

Per 128-dest tile: gather source rows with dma_gather (int16 idx, bf16 512B
rows), build selector S[e,d] = w_e * (dest_e == d) with one tensor_scalar per
128-edge chunk, PSUM-accumulate matmuls. Phase 1 computes T1 transposed
directly (lhsT=G feature halves, rhs=S) so no PE transposes are needed.
Between passes, U (bf16) is AllGather'd (two collectives, first pipelined
behind pass 1).

dma_gather descriptor generation runs on ONE Q7 core-pair selected by
queue_num (4 SWDGE queues -> 4-way desc-gen parallelism, ~2ns/idx aggregate
vs ~8ns/idx on one queue). To keep all 4 queues busy the gather stream is
deep-pipelined: section-granular gathers into a deep buffer pool, ragged
per-tile static index counts (sections padded with index 0 / weight 0 only
up to their own 128-multiple, not a global max), strict round-robin queues.
meta/idx DRAM layouts stay row-blocked (contiguous per tile) so their loads
coalesce into spray descriptors.
"""

import numpy as np
import ml_dtypes

import concourse.bass as bass
import concourse.mybir as mybir
import concourse.tile as tile
from concourse import bacc
from concourse.bass_utils import run_bass_kernel_spmd

N = 50000
NP = 50176          # padded: 8 * 49 * 128
F = 256
P = 128
NCORE = 8
SH = NP // NCORE    # 6272 rows per core
T = SH // P         # 49 dest tiles per core
TA = 24             # tiles whose U rows go to the first AllGather
TB = T - TA         # 25
SHA = TA * P        # 3072
SHB = TB * P        # 3200
XSPLIT = NP // 2    # 25088: phase-1 gather table split (int16 range)

F32 = mybir.dt.float32
BF16 = mybir.dt.bfloat16
I16 = mybir.dt.int16
NPBF16 = ml_dtypes.bfloat16


# ---------------------------------------------------------------- host prep

def _wrap_idx(arr):
    """[n] int16 -> [128, n//16]: 16-partition wrap, replicated for 8 Q7 cores."""
    n = len(arr)
    w16 = arr.reshape(n // 16, 16).T
    return np.tile(w16, (8, 1)).copy()


def _pad_to(arr, n, fill):
    out = np.full(n, fill, arr.dtype)
    out[: len(arr)] = arr
    return out


def prepare(x, edge_row, edge_col, edge_w, weight, bias, mode="full", repeat=1,
            pool_every=1000000, nq=4, gbufs=8):
    x = np.asarray(x, np.float32)
    edge_row = np.asarray(edge_row, np.int32)
    edge_col = np.asarray(edge_col, np.int32)
    edge_w = np.asarray(edge_w, np.float32)
    weight = np.asarray(weight, np.float32)
    bias = np.asarray(bias, np.float32)

    order = np.argsort(edge_row, kind="stable")
    edge_row = edge_row[order]
    edge_col = edge_col[order]
    edge_w = edge_w[order]

    x_pad = np.zeros((NP, F), np.float32)
    x_pad[:N] = x

    bounds = np.searchsorted(edge_row, np.arange(0, NP + 1, P))

    owner = edge_col // SH
    rloc = edge_col % SH
    m1 = edge_col < XSPLIT
    i1lo = edge_col
    i1hi = edge_col - XSPLIT
    m2 = rloc < SHA
    i2lo = owner * SHA + rloc
    i2hi = owner * SHB + (rloc - SHA)

    dloc = (edge_row % P).astype(np.float32)

    def cnt_of(n):
        return max(1, -(-int(n) // P))

    # per-tile static section sizes = max over cores (one shared program)
    geom = np.zeros((NCORE, T, 4), np.int32)
    for c in range(NCORE):
        for t in range(T):
            g = c * T + t
            lo, hi = bounds[g], bounds[g + 1]
            msk1 = m1[lo:hi]
            msk2 = m2[lo:hi]
            geom[c, t] = (cnt_of(msk1.sum()), cnt_of((~msk1).sum()),
                          cnt_of(msk2.sum()), cnt_of((~msk2).sum()))
    geom_max = [tuple(int(v) for v in geom[:, t].max(axis=0))
                for t in range(T)]
    CS = max(max(gm) for gm in geom_max)
    C1W = max(gm[0] + gm[1] for gm in geom_max)   # phase-1 tile width
    C2AW = max(gm[2] for gm in geom_max)
    C2BW = max(gm[3] for gm in geom_max)

    def build_sec(vals_i, d_all, w_all, cnt):
        """One section padded to cnt chunks: meta [128, 2*cnt], idx [128, cnt*8]."""
        idx = _pad_to(vals_i.astype(np.int16), cnt * P, 0)
        d_sec = _pad_to(d_all, cnt * P, 0.0)
        w_sec = _pad_to(w_all, cnt * P, 0.0)
        meta = np.concatenate(
            [d_sec.reshape(cnt, P).T, w_sec.reshape(cnt, P).T], axis=1
        ).astype(NPBF16)
        return meta, _wrap_idx(idx), w_sec.reshape(cnt, P).T.astype(np.float32)

    # fixed-width row-blocked layouts (contiguous per tile -> spray DMA);
    # sections packed at the front of each row-block, ragged sizes used
    # only for the gather instruction shapes
    meta1 = np.zeros((NCORE, T, P, 2 * C1W), NPBF16)
    idx1 = np.zeros((NCORE, T, P, C1W * 8), np.int16)
    meta2a = np.zeros((NCORE, T, P, 2 * C2AW), NPBF16)
    idx2a = np.zeros((NCORE, T, P, C2AW * 8), np.int16)
    meta2b = np.zeros((NCORE, T, P, 2 * C2BW), NPBF16)
    idx2b = np.zeros((NCORE, T, P, C2BW * 8), np.int16)
    wm1 = np.zeros((NCORE, T, P, C1W), np.float32)
    wm2a = np.zeros((NCORE, T, P, C2AW), np.float32)
    wm2b = np.zeros((NCORE, T, P, C2BW), np.float32)

    for c in range(NCORE):
        for t in range(T):
            g = c * T + t
            lo, hi = bounds[g], bounds[g + 1]
            sel = slice(lo, hi)
            d_all = dloc[sel]
            w_all = edge_w[sel]
            msk1 = m1[sel]
            msk2 = m2[sel]
            cl, ch, ca, cb = geom_max[t]
            C = cl + ch
            mlo, xlo, wlo = build_sec(i1lo[sel][msk1], d_all[msk1],
                                      w_all[msk1], cl)
            mhi, xhi, whi = build_sec(i1hi[sel][~msk1], d_all[~msk1],
                                      w_all[~msk1], ch)
            # combined layout: [d_lo | d_hi | w_lo | w_hi], [idx_lo | idx_hi]
            meta1[c, t, :, 0:C] = np.concatenate(
                [mlo[:, :cl], mhi[:, :ch]], axis=1)
            meta1[c, t, :, C:2 * C] = np.concatenate(
                [mlo[:, cl:], mhi[:, ch:]], axis=1)
            idx1[c, t, :, 0:C * 8] = np.concatenate([xlo, xhi], axis=1)
            wm1[c, t, :, 0:C] = np.concatenate([wlo, whi], axis=1)

            ma, xa, wa = build_sec(i2lo[sel][msk2], d_all[msk2],
                                   w_all[msk2], ca)
            meta2a[c, t, :, 0:2 * ca] = ma
            idx2a[c, t, :, 0:ca * 8] = xa
            wm2a[c, t, :, 0:ca] = wa
            mb, xb, wb = build_sec(i2hi[sel][~msk2], d_all[~msk2],
                                   w_all[~msk2], cb)
            meta2b[c, t, :, 0:2 * cb] = mb
            idx2b[c, t, :, 0:cb * 8] = xb
            wm2b[c, t, :, 0:cb] = wb

    # xT tiles: [t*128, 256]; cols k*128+j hold x_pad[tile row j, feat k*128+p]
    xl = x_pad.reshape(NCORE, T, P, F)
    xtt = np.ascontiguousarray(np.transpose(xl, (0, 1, 3, 2))).reshape(
        NCORE, T, 2, P, P)
    xtt = np.ascontiguousarray(np.transpose(xtt, (0, 1, 3, 2, 4))).reshape(
        NCORE, T, P, F)

    # weights: m0 = W0 - W2, m1 = W1, m2 = 2*W2; wpack[p, (k*3+m)*256 + j]
    wm = np.stack([weight[0] - weight[2], weight[1], 2.0 * weight[2]])
    wpack = np.empty((P, 2 * 3 * F), np.float32)
    for k in range(2):
        for m in range(3):
            wpack[:, (k * 3 + m) * F:(k * 3 + m + 1) * F] = \
                wm[m, k * P:(k + 1) * P, :]

    biasrow = bias.reshape(1, F)
    iota = np.broadcast_to(np.arange(P, dtype=np.float32), (P, P))
    x_lo = x_pad[:XSPLIT]
    x_hi = x_pad[XSPLIT:]

    nc = build_program(geom_max, CS, C1W, C2AW, C2BW, mode=mode,
                       repeat=repeat, nq=nq, gbufs=gbufs)

    in_maps = []
    for c in range(NCORE):
        in_maps.append({
            "x_lo": x_lo.astype(NPBF16),
            "x_hi": x_hi.astype(NPBF16),
            "meta1": np.ascontiguousarray(meta1[c].reshape(T * P, 2 * C1W)),
            "idx1": np.ascontiguousarray(idx1[c].reshape(T * P, C1W * 8)),
            "meta2a": np.ascontiguousarray(meta2a[c].reshape(T * P, 2 * C2AW)),
            "idx2a": np.ascontiguousarray(idx2a[c].reshape(T * P, C2AW * 8)),
            "meta2b": np.ascontiguousarray(meta2b[c].reshape(T * P, 2 * C2BW)),
            "idx2b": np.ascontiguousarray(idx2b[c].reshape(T * P, C2BW * 8)),
            "wm1": np.ascontiguousarray(wm1[c].reshape(T * P, C1W)),
            "wm2a": np.ascontiguousarray(wm2a[c].reshape(T * P, C2AW)),
            "wm2b": np.ascontiguousarray(wm2b[c].reshape(T * P, C2BW)),
            "xtt": np.ascontiguousarray(xtt[c].reshape(T * P, F)).astype(NPBF16),
            "wpack": wpack.astype(NPBF16),
            "biasrow": biasrow.astype(NPBF16),
            "iota": iota.astype(NPBF16),
        })
    return nc, in_maps


def assemble(results):
    out = np.concatenate([results[c]["out"] for c in range(NCORE)], axis=0)
    return np.ascontiguousarray(out[:N])


# ---------------------------------------------------------------- device

def build_program(geom, CS, C1W, C2AW, C2BW, mode="full", repeat=1, nq=4,
                  gbufs=8):
    # geom: per-tile (c1l, c1h, c2a, c2b) static section sizes
    # mode: "full" | "p1" (phase 1 only) | "p1cc" (phase 1 + collectives)
    nc = bacc.Bacc("TRN2", target_bir_lowering=False, debug=False,
                   num_devices=NCORE, num_swdge_queues=nq)

    x_lo_d = nc.dram_tensor("x_lo", [XSPLIT, F], BF16, kind="ExternalInput")
    x_hi_d = nc.dram_tensor("x_hi", [NP - XSPLIT, F], BF16,
                            kind="ExternalInput")
    meta1_d = nc.dram_tensor("meta1", [T * P, 2 * C1W], BF16,
                             kind="ExternalInput")
    idx1_d = nc.dram_tensor("idx1", [T * P, C1W * 8], I16,
                            kind="ExternalInput")
    meta2a_d = nc.dram_tensor("meta2a", [T * P, 2 * C2AW], BF16,
                              kind="ExternalInput")
    idx2a_d = nc.dram_tensor("idx2a", [T * P, C2AW * 8], I16,
                             kind="ExternalInput")
    meta2b_d = nc.dram_tensor("meta2b", [T * P, 2 * C2BW], BF16,
                              kind="ExternalInput")
    idx2b_d = nc.dram_tensor("idx2b", [T * P, C2BW * 8], I16,
                             kind="ExternalInput")
    wm1_d = nc.dram_tensor("wm1", [T * P, C1W], F32, kind="ExternalInput")
    wm2a_d = nc.dram_tensor("wm2a", [T * P, C2AW], F32, kind="ExternalInput")
    wm2b_d = nc.dram_tensor("wm2b", [T * P, C2BW], F32, kind="ExternalInput")
    xtt_d = nc.dram_tensor("xtt", [T * P, F], BF16, kind="ExternalInput")
    wpack_d = nc.dram_tensor("wpack", [P, 6 * F], BF16, kind="ExternalInput")
    bias_d = nc.dram_tensor("biasrow", [1, F], BF16, kind="ExternalInput")
    iota_d = nc.dram_tensor("iota", [P, P], BF16, kind="ExternalInput")
    out_d = nc.dram_tensor("out", [SH, F], F32, kind="ExternalOutput")

    gq_counter = [0]

    with tile.TileContext(nc) as tc:
        with tc.tile_pool(name="const", bufs=1) as cp, \
             tc.tile_pool(name="mp", bufs=8) as mp, \
             tc.tile_pool(name="gp", bufs=gbufs) as gp, \
             tc.tile_pool(name="sb", bufs=3) as sb, \
             tc.tile_pool(name="sel", bufs=3) as selp, \
             tc.tile_pool(name="stash", bufs=T) as stash, \
             tc.tile_pool(name="ps", bufs=2, space="PSUM") as ps, \
             tc.tile_pool(name="dram", bufs=1, space="DRAM") as dp:

            iota_t = cp.tile([P, P], BF16, tag="iota")
            nc.sync.dma_start(out=iota_t[:], in_=iota_d[:])
            wpk = cp.tile([P, 6 * F], BF16, tag="wpk")
            nc.sync.dma_start(out=wpk[:], in_=wpack_d[:])
            bias_t = cp.tile([1, F], BF16, tag="bias")
            nc.sync.dma_start(out=bias_t[:], in_=bias_d[:])
            ones_t = cp.tile([1, P], BF16, tag="ones")
            nc.vector.memset(ones_t[:], 1.0)

            def next_q():
                q = gq_counter[0] % nq
                gq_counter[0] += 1
                return q

            def w_ap(m, k):
                return wpk[:, (k * 3 + m) * F:(k * 3 + m + 1) * F]

            CSW = max(geom_t[0] + geom_t[1] for geom_t in geom)

            def build_sel_wide(meta_t, wm_t, C):
                """S[e, c, d] = (iota[d] == dest[e,c]) * w[e,c].

                is_equal as one wide tensor_tensor on DVE (1-port mode: never
                locks GpSimd out of SBUF, unlike tensor_scalar's 2-port mode
                which starves SWDGE descriptor generation). The w-multiply is
                split: first chunks as a wide DVE tensor_tensor, the rest as
                per-chunk ACT copies with per-partition scale (ACT never
                contends with anyone)."""
                s_w = selp.tile([P, CSW, P], BF16, tag="s")
                in0 = iota_t[:].unsqueeze(1).broadcast_to([P, C, P])
                d_bc = meta_t[:, 0:C].unsqueeze(2).broadcast_to([P, C, P])
                nc.vector.tensor_tensor(out=s_w[:, 0:C, :], in0=in0, in1=d_bc,
                                        op=mybir.AluOpType.is_equal)
                K = C - (2 * C) // 7
                if K > 0:
                    w_bc = meta_t[:, C:C + K].unsqueeze(2).broadcast_to(
                        [P, K, P])
                    nc.vector.tensor_tensor(out=s_w[:, 0:K, :],
                                            in0=s_w[:, 0:K, :],
                                            in1=w_bc, op=mybir.AluOpType.mult)
                for c in range(K, C):
                    nc.scalar.mul(s_w[:, c, :], s_w[:, c, :],
                                  wm_t[:, c:c + 1])
                return s_w

            def load_meta(t, meta_d, mw, idx_d, xw, wm_d):
                meta_t = mp.tile([P, 2 * mw], BF16, tag="meta")
                idx_t = mp.tile([P, xw * 8], I16, tag="idx")
                wm_t = mp.tile([P, mw], F32, tag="wm")
                nc.sync.dma_start(out=meta_t[:],
                                  in_=meta_d[t * P:(t + 1) * P, :])
                nc.sync.dma_start(out=idx_t[:],
                                  in_=idx_d[t * P:(t + 1) * P, :])
                nc.sync.dma_start(out=wm_t[:],
                                  in_=wm_d[t * P:(t + 1) * P, :])
                return meta_t, idx_t, wm_t

            def gather(idx_t, ioff, cnt, tab):
                g = gp.tile([P, CS, F], BF16, tag="g")
                nc.gpsimd.dma_gather(
                    out_ap=g[:, 0:cnt, :], in_ap=tab[:, :],
                    idxs_ap=idx_t[:, ioff:ioff + cnt * 8],
                    num_idxs=cnt * P, num_idxs_reg=cnt * P, elem_size=F,
                    single_packet=False, queue_num=next_q())
                return g

            for _rep in range(repeat):
                u_a = dp.tile([SHA, F], BF16, tag=f"ua{_rep}")
                u_b = dp.tile([SHB, F], BF16, tag=f"ub{_rep}")
                u_g1 = dp.tile([NCORE * SHA, F], BF16, tag=f"ug1{_rep}",
                               addr_space="Shared")
                u_g2 = dp.tile([NCORE * SHB, F], BF16, tag=f"ug2{_rep}",
                               addr_space="Shared")
                o1_tiles = []
                if mode == "p2x":
                    for t in range(T):
                        o1 = stash.tile([P, F], F32, tag="o1")
                        nc.scalar.memzero(o1[:])
                        o1_tiles.append(o1)
                # ---------------- phase 1 ----------------
                # mode g1: gathers only (Pool/DMA throughput in situ)
                # mode s1: selectors+matmuls only (compute chain in situ)
                for t in ([] if mode == "p2x" else range(T)):
                    cl, ch = geom[t][0], geom[t][1]
                    C = cl + ch
                    meta_t, idx_t, wm_t = load_meta(t, meta1_d, C1W,
                                                    idx1_d, C1W, wm1_d)
                    if mode == "g1":
                        gather(idx_t, 0, cl, x_lo_d)
                        gather(idx_t, cl * 8, ch, x_hi_d)
                        continue
                    if mode == "gs1":
                        g_lo = gather(idx_t, 0, cl, x_lo_d)
                        g_hi = gather(idx_t, cl * 8, ch, x_hi_d)
                        build_sel_wide(meta_t, wm_t, C)
                        continue
                    if mode == "s1":
                        g_lo = gp.tile([P, CS, F], BF16, tag="g")
                        g_hi = gp.tile([P, CS, F], BF16, tag="g")
                        nc.vector.memset(g_lo[:, 0, 0:2], 0.0)
                        nc.vector.memset(g_hi[:, 0, 0:2], 0.0)
                    else:
                        g_lo = gather(idx_t, 0, cl, x_lo_d)
                        g_hi = gather(idx_t, cl * 8, ch, x_hi_d)
                    # T1.T accumulated directly: two [128f, 128d] PSUM tiles
                    # (separate banks — interleaved accumulation groups
                    # sharing one bank corrupt results)
                    t1T = sb.tile([P, F], BF16, tag="t1T")
                    t1a_ps = ps.tile([P, P], F32, tag="t1a")
                    t1b_ps = ps.tile([P, P], F32, tag="t1b")
                    halves = [(t1a_ps[:], slice(0, P)),
                              (t1b_ps[:], slice(P, F))]
                    s_w = build_sel_wide(meta_t, wm_t, C)
                    for c in range(C):
                        g, ci = (g_lo, c) if c < cl else (g_hi, c - cl)
                        for acc_ap, fsl in halves:
                            nc.tensor.matmul(acc_ap, lhsT=g[:, ci, fsl],
                                             rhs=s_w[:, c, :],
                                             start=(c == 0), stop=(c == C - 1))
                    for acc_ap, fsl in halves:
                        nc.scalar.copy(t1T[:, fsl], acc_ap)

                    u_ps = ps.tile([P, F], F32, tag="uo2")
                    nc.tensor.matmul(u_ps[:], lhsT=t1T[:, 0:P], rhs=w_ap(2, 0),
                                     start=True, stop=False)
                    nc.tensor.matmul(u_ps[:], lhsT=t1T[:, P:F], rhs=w_ap(2, 1),
                                     start=False, stop=True)
                    u_sb = sb.tile([P, F], BF16, tag="usb")
                    nc.scalar.copy(u_sb[:], u_ps[:])
                    if t < TA:
                        nc.scalar.dma_start(out=u_a[t * P:(t + 1) * P, :],
                                            in_=u_sb[:])
                    else:
                        nc.scalar.dma_start(
                            out=u_b[(t - TA) * P:(t - TA + 1) * P, :],
                            in_=u_sb[:])

                    xt_t = sb.tile([P, F], BF16, tag="xt")
                    nc.sync.dma_start(out=xt_t[:],
                                      in_=xtt_d[t * P:(t + 1) * P, :])
                    o_ps = ps.tile([P, F], F32, tag="o")
                    nc.tensor.matmul(o_ps[:], lhsT=xt_t[:, 0:P], rhs=w_ap(0, 0),
                                     start=True, stop=False)
                    nc.tensor.matmul(o_ps[:], lhsT=xt_t[:, P:F], rhs=w_ap(0, 1),
                                     start=False, stop=False)
                    nc.tensor.matmul(o_ps[:], lhsT=t1T[:, 0:P], rhs=w_ap(1, 0),
                                     start=False, stop=False)
                    nc.tensor.matmul(o_ps[:], lhsT=t1T[:, P:F], rhs=w_ap(1, 1),
                                     start=False, stop=False)
                    nc.tensor.matmul(o_ps[:], lhsT=ones_t[:], rhs=bias_t[:],
                                     start=False, stop=True)
                    o1 = stash.tile([P, F], F32, tag="o1")
                    nc.scalar.copy(o1[:], o_ps[:])
                    o1_tiles.append(o1)

                    if t == TA - 1 and mode not in ("p1",):
                        nc.gpsimd.collective_compute(
                            "AllGather", mybir.AluOpType.bypass,
                            replica_groups=[list(range(NCORE))],
                            ins=[u_a[:].opt()], outs=[u_g1[:].opt()])
                if mode not in ("p1",):
                    nc.gpsimd.collective_compute(
                        "AllGather", mybir.AluOpType.bypass,
                        replica_groups=[list(range(NCORE))],
                        ins=[u_b[:].opt()], outs=[u_g2[:].opt()])

                # ---------------- phase 2a: a sections (need only AG1) ----
                if mode in ("full", "p2x"):
                    for t in range(T):
                        c2 = geom[t][2]
                        meta_t, idx_t, wm_t = load_meta(
                            t, meta2a_d, C2AW, idx2a_d, C2AW, wm2a_d)
                        g = gather(idx_t, 0, c2, u_g1)
                        o2_ps = ps.tile([P, F], F32, tag="uo2")
                        s_w = build_sel_wide(meta_t, wm_t, c2)
                        for c in range(c2):
                            nc.tensor.matmul(o2_ps[:], lhsT=s_w[:, c, :],
                                             rhs=g[:, c, :],
                                             start=(c == 0),
                                             stop=(c == c2 - 1))
                        nc.vector.tensor_add(o1_tiles[t][:], o1_tiles[t][:],
                                             o2_ps[:])
                    # ------------ phase 2b: b sections (need AG2) ----------
                    for t in range(T):
                        c2 = geom[t][3]
                        meta_t, idx_t, wm_t = load_meta(
                            t, meta2b_d, C2BW, idx2b_d, C2BW, wm2b_d)
                        g = gather(idx_t, 0, c2, u_g2)
                        o2_ps = ps.tile([P, F], F32, tag="uo2")
                        s_w = build_sel_wide(meta_t, wm_t, c2)
                        for c in range(c2):
                            nc.tensor.matmul(o2_ps[:], lhsT=s_w[:, c, :],
                                             rhs=g[:, c, :],
                                             start=(c == 0),
                                             stop=(c == c2 - 1))
                        fin = sb.tile([P, F], F32, tag="fin")
                        nc.vector.tensor_add(fin[:], o1_tiles[t][:], o2_ps[:])
                        nc.scalar.dma_start(out=out_d[t * P:(t + 1) * P, :],
                                            in_=fin[:])
                else:
                    for t in range(T):
                        fin = sb.tile([P, F], F32, tag="fin")
                        if o1_tiles:
                            nc.scalar.copy(fin[:], o1_tiles[t][:])
                        else:
                            nc.scalar.memzero(fin[:])
                        nc.scalar.dma_start(out=out_d[t * P:(t + 1) * P, :],
                                            in_=fin[:])

    nc.compile()
    return nc


# ---------------------------------------------------------------- entry

def kernel(x, edge_row, edge_col, edge_w, weight, bias):
    nc, in_maps = prepare(x, edge_row, edge_col, edge_w, weight, bias)
    res = run_bass_kernel_spmd(nc, in_maps, core_ids=list(range(NCORE)))
    return assemble(res.results)
